# revision 14
# baseline (speedup 1.0000x reference)
"""DiT block with block-diffusion sparse attention on 8 Trainium2 NeuronCores.

v2 strategy (vs v1 baseline):
  - adaLN modulation computed on HOST (tiny matvec); gamma scales folded into
    the QKV / MLP1 weights, shifts folded into bias rows. No device adaLN.
  - LN1 never materializes h: QKV matmuls read host-pretransposed x^T
    directly; the per-token (-mu, sigma) correction enters the same PSUM as a
    rank-2 matmul with stationary (u, b) = ((W*gamma)@1, W@beta); the 1/sigma
    scale is folded into the RoPE cos/sin tables. Stats come from a separate
    token-major bn_stats pass (DMA-parallel, off the critical path).
  - QKV + RoPE + attention: head-parallel, 2 heads per core, scores in S^T
    orientation with a ones-column in V for the softmax denominator; only the
    non-masked block-sparse k ranges are computed.
  - One AllToAll converts head-sharded attention output to token-sharded.
  - attn_out, LN2, MLP run token-sharded (256 tokens/core) with full bf16
    weights, prefetched during the attention/A2A window. LN2 row broadcasts
    are PE rank-1 matmuls into PSUM.
  - Output written feature-major; host re-transposes.
All matmuls bf16 with fp32 accumulation; norm/softmax stats in fp32.
"""

import os
import numpy as np
import ml_dtypes

import concourse.bass as bass
import concourse.tile as tile
from concourse import bacc, mybir
from concourse.bass_utils import run_bass_kernel_spmd
from concourse.masks import make_identity

bf16 = ml_dtypes.bfloat16
FP = mybir.dt.float32
BF = mybir.dt.bfloat16
AF = mybir.ActivationFunctionType
ALU = mybir.AluOpType

NCORES = 8
S, N, D, H, HD, BS, COND = 2048, 1024, 1024, 16, 64, 16, 128
TOK = S // NCORES  # 256 tokens per core after A2A


def _attn_schedule():
    """Per q-chunk list of (ktile, col0, col1, mask) in S^T orientation."""
    sched = []
    for c in range(4):
        items = []
        if c < 2:  # noisy q chunk
            for j in range(4 * c + 4):  # clean k tiles, bq > bk
                js = j - 4 * c
                if js < 0:
                    items.append((8 + j, 0, 512, None))
                else:
                    items.append((8 + j, 128 * js, 512, "strict"))
            for s in range(4):  # own-block diagonal (noisy k)
                items.append((4 * c + s, 128 * s, 128 * s + 128, "diag"))
        else:  # clean q chunk, bq >= bk
            cq = c - 2
            for j in range(4 * cq + 4):
                js = j - 4 * cq
                if js < 0:
                    items.append((8 + j, 0, 512, None))
                else:
                    items.append((8 + j, 128 * js, 512, "incl"))
        assert items[0][1] == 0 and items[0][2] == 512
        sched.append(items)
    return sched


MASK_OFF = {"diag": 0, "strict": 128, "incl": 256}


def build_program(single=False):
    """single=True builds a 1-device variant (A2A replaced by a local DMA
    copy) for TimelineSim cost-model analysis."""
    nc = bacc.Bacc(
        "TRN2", target_bir_lowering=False, debug=False,
        enable_asserts=False, num_devices=1 if single else NCORES,
    )

    def din(name, shape, dt=BF):
        return nc.dram_tensor(name, shape, dt, kind="ExternalInput").ap()

    x_d = din("x", [S, D])                            # token-major (stats)
    xT_d = din("xT", [8, 128, S])                     # feature-major (k, p, t)
    xsT_d = din("xsliceT", [8, 128, TOK])             # residual slice (k, p, t)
    trig_d = din("trig", [128, 2 * S])                # cos2 | sin2(signed)
    mask01_d = din("mask01", [128, 384])              # diag | strict | incl
    wqkv_d = din("wqkvT", [3, 128, 8, 128])           # (s, p, k, c) g-scaled
    ub_d = din("ubrow", [2, 384])                     # (u; b) per-core slice
    wao_d = din("waoT", [2, 128, 4, 8, 128])          # (g, p, mi, k, c)
    w1_d = din("w1T", [8, 128, 4, 8, 128])            # (g, p, mi, k, c) scaled
    w2_d = din("w2T", [8, 128, 32, 128])              # (m, p, k, c)
    smallc_d = din("smallc", [128, 56], FP)           # gmsa|gmlp|b1'|b2
    out_d = nc.dram_tensor("out", [8, 128, TOK], FP, kind="ExternalOutput").ap()

    sched = _attn_schedule()

    with tile.TileContext(nc) as tc:
        with tc.tile_pool(name="const", bufs=1) as const, \
             tc.tile_pool(name="dram", bufs=1, space="DRAM") as dram, \
             tc.tile_pool(name="qkvr", bufs=1) as qkvr, \
             tc.tile_pool(name="vaugp", bufs=1) as vaugp, \
             tc.tile_pool(name="x2p", bufs=1) as x2p, \
             tc.tile_pool(name="gp", bufs=1) as gp:

            # ---------------- constants / small inputs ----------------
            trig_sb = const.tile([128, 2 * S], BF)
            nc.sync.dma_start(out=trig_sb, in_=trig_d)
            mask_sb = const.tile([128, 384], BF)
            nc.sync.dma_start(out=mask_sb, in_=mask01_d)
            smallc = const.tile([128, 56], FP)
            nc.sync.dma_start(out=smallc, in_=smallc_d)
            gmsa_sb = smallc[:, 0:8]
            gmlp_sb = smallc[:, 8:16]
            b1_sb = smallc[:, 16:48]
            b2_sb = smallc[:, 48:56]
            ub_sb = const.tile([2, 384], BF)
            nc.sync.dma_start(out=ub_sb, in_=ub_d)
            ones_sb = const.tile([128, 1], BF)
            nc.vector.memset(ones_sb, 1.0)
            ones_row = const.tile([1, 128], BF)
            nc.vector.memset(ones_row, 1.0)
            eps128 = const.tile([128, 1], FP)
            nc.vector.memset(eps128, 1e-5)
            eps1 = const.tile([1, 1], FP)
            nc.vector.memset(eps1, 1e-5)
            ident_f = const.tile([128, 128], FP)
            make_identity(nc, ident_f)
            ident_b = const.tile([128, 128], BF)
            nc.vector.tensor_copy(out=ident_b, in_=ident_f)

            # residual slice (feature-major) straight from DRAM
            xsT = [x2p.tile([128, TOK], BF, name=f"xsT{j}") for j in range(8)]
            for fj in range(8):
                nc.sync.dma_start(out=xsT[fj], in_=xsT_d[fj])

            # ---------------- phase 0: LN1 stats (token-major) ---------
            # per 128-token tile: bn stats -> (-mu, sd, rstd) columns of a
            # [128, 48] tile; one PE transpose + DRAM bounce turns them into
            # [3, 2048] rows (negmu | sd | rstd) aligned with qT columns.
            stats_dr = dram.tile([3, S], BF)
            rows_sb = const.tile([2, S], BF)   # (negmu ; sd)
            rstd_row = const.tile([1, S], BF)
            with tc.tile_pool(name="xstat", bufs=2) as xstat, \
                 tc.tile_pool(name="statp", bufs=4) as statp, \
                 tc.tile_pool(name="st48", bufs=1) as st48p, \
                 tc.tile_pool(name="stps", bufs=1, space="PSUM") as stps:
                stat48 = st48p.tile([128, 48], FP)
                x_r = x_d.rearrange("(t p) d -> p t d", p=128)  # [128,16,D]
                for g in range(4):
                    xg = xstat.tile([128, 4, D], BF, tag="x")
                    nc.sync.dma_start(out=xg, in_=x_r[:, 4 * g:4 * g + 4, :])
                    for sub in range(4):
                        ti = 4 * g + sub
                        x_sb = xg[:, sub, :]
                        st = statp.tile([128, 2, 6], FP, tag="bst")
                        for sg in range(2):
                            nc.vector.bn_stats(
                                out=st[:, sg, :],
                                in_=x_sb[:, 512 * sg:512 * sg + 512])
                        mv = statp.tile([128, 2], FP, tag="mv")
                        nc.vector.bn_aggr(out=mv, in_=st)
                        # negmu / sd / rstd straight into stat48 columns
                        nc.vector.tensor_scalar_mul(
                            stat48[:, ti:ti + 1], mv[:, 0:1], -1.0)
                        nc.scalar.activation(
                            out=stat48[:, 16 + ti:17 + ti], in_=mv[:, 1:2],
                            func=AF.Sqrt, bias=eps128, scale=1.0)
                        nc.vector.reciprocal(
                            out=stat48[:, 32 + ti:33 + ti],
                            in_=stat48[:, 16 + ti:17 + ti])
                ps = stps.tile([48, 128], FP)
                nc.tensor.transpose(ps, stat48, ident_f)
                st48b = st48p.tile([48, 128], BF)
                nc.vector.tensor_copy(out=st48b, in_=ps)
                nc.sync.dma_start(
                    out=stats_dr.rearrange("v (t p) -> (v t) p", p=128),
                    in_=st48b)
                nc.sync.dma_start(out=rows_sb, in_=stats_dr[0:2])
                nc.sync.dma_start(out=rstd_row, in_=stats_dr[2:3])

            # rstd broadcast to [128, S] via PE rank-1; cosr/sinr fold 1/sd
            cosr = const.tile([128, S], BF)
            sinr = const.tile([128, S], BF)
            with tc.tile_pool(name="rbps", bufs=1, space="PSUM") as rbps:
                rstd_ps = rbps.tile([128, S], FP)
                for q in range(4):
                    nc.tensor.matmul(
                        rstd_ps[:, 512 * q:512 * q + 512], ones_row,
                        rstd_row[:, 512 * q:512 * q + 512],
                        start=True, stop=True, skip_group_check=True)
                nc.vector.tensor_mul(cosr, trig_sb[:, 0:S], rstd_ps)
                nc.vector.tensor_mul(sinr, trig_sb[:, S:2 * S], rstd_ps)

            # ---------------- phase 1: QKV + RoPE ----------------------
            qT = qkvr.tile([128, S], BF)
            kT = qkvr.tile([128, S], BF)
            vT = qkvr.tile([128, S], BF)
            qkv_dst = [qT, kT, vT]

            with tc.tile_pool(name="xTp", bufs=1) as xTp, \
                 tc.tile_pool(name="wqkvp", bufs=1) as wqkvp, \
                 tc.tile_pool(name="ropep", bufs=3) as ropep, \
                 tc.tile_pool(name="qkvps", bufs=3, space="PSUM") as qkvps:
                xT_sb = [xTp.tile([128, S], BF, name=f"xT{k}")
                         for k in range(8)]
                for k in range(8):
                    nc.sync.dma_start(out=xT_sb[k], in_=xT_d[k])
                wq_sb = [wqkvp.tile([128, 8 * 128], BF, name=f"wq{m}")
                         for m in range(3)]
                for m in range(3):
                    nc.scalar.dma_start(
                        out=wq_sb[m].rearrange("p (k c) -> p k c", c=128),
                        in_=wqkv_d[m])

                for n in range(4):
                    nsl = slice(512 * n, 512 * n + 512)
                    for m in range(3):
                        ps = qkvps.tile([128, 512], FP, tag="qkvps")
                        for k in range(8):
                            nc.tensor.matmul(
                                ps, wq_sb[m][:, 128 * k:128 * k + 128],
                                xT_sb[k][:, nsl],
                                start=(k == 0), stop=False)
                        # rank-2: + u (.) (-mu)  +  b (.) sd
                        nc.tensor.matmul(
                            ps, ub_sb[:, 128 * m:128 * m + 128],
                            rows_sb[0:2, nsl], start=False, stop=True,
                            skip_group_check=True)
                        # rope evac: dst = ps*cosr + rot(ps)*sinr
                        # (1/sd folded into cosr/sinr)
                        t1 = ropep.tile([128, 512], BF, tag="t1")
                        nc.vector.tensor_mul(t1, ps, cosr[:, nsl])
                        t2 = ropep.tile([128, 512], BF, tag="t2")
                        for h in range(2):
                            r = 64 * h
                            nc.vector.tensor_mul(
                                t2[r:r + 32, :], ps[r + 32:r + 64, :],
                                sinr[r + 32:r + 64, nsl])
                            nc.vector.tensor_mul(
                                t2[r + 32:r + 64, :], ps[r:r + 32, :],
                                sinr[r:r + 32, nsl])
                        nc.vector.tensor_add(
                            qkv_dst[m][:, nsl], t1, t2)

            # ---------------- phase 2: V token-major (+ones col) -------
            vaug = [vaugp.tile([128, 130], BF, name=f"vaug{kt}")
                    for kt in range(16)]
            with tc.tile_pool(name="vtps", bufs=2, space="PSUM") as vtps:
                for kt in range(16):
                    ps = vtps.tile([128, 128], BF, tag="vt")
                    nc.tensor.transpose(
                        ps, vT[:, 128 * kt:128 * kt + 128], ident_b)
                    va = vaug[kt]
                    nc.vector.memset(va[:, 64:65], 1.0)
                    nc.vector.memset(va[:, 129:130], 1.0)
                    nc.scalar.copy(
                        out=va[:, 0:130].rearrange(
                            "p (h y) -> p h y", y=65)[:, :, 0:64],
                        in_=ps.rearrange("p (h d) -> p h d", d=64))

            # ---------------- weight prefetch (runs under attn + A2A) --
            waop = tc.alloc_tile_pool(name="waop", bufs=2)
            w1p = tc.alloc_tile_pool(name="w1p", bufs=5)
            w2p = tc.alloc_tile_pool(name="w2p", bufs=4)
            wao_sb = [waop.tile([128, 4 * 8 * 128], BF, tag="wao",
                                name=f"wao{g}")
                      for g in range(2)]
            for g in range(2):
                nc.scalar.dma_start(
                    out=wao_sb[g].rearrange("p (mi k c) -> p mi k c",
                                            k=8, c=128),
                    in_=wao_d[g])
            w1_sb = [w1p.tile([128, 4 * 8 * 128], BF, tag="w1",
                              name=f"w1_{g}")
                     for g in range(8)]
            for g in range(8):
                nc.scalar.dma_start(
                    out=w1_sb[g].rearrange("p (mi k c) -> p mi k c",
                                           k=8, c=128),
                    in_=w1_d[g])
            w2_sb = [w2p.tile([128, 32 * 128], BF, tag="w2",
                              name=f"w2_{m}")
                     for m in range(8)]
            for m in range(8):
                nc.scalar.dma_start(
                    out=w2_sb[m].rearrange("p (k c) -> p k c", c=128),
                    in_=w2_d[m])

            # ---------------- phase 3: sparse attention ----------------
            # onorm split by q-half; ONE AllToAll ships both halves. Core j
            # owns the contiguous token block [256j, 256j+256).
            onorm = [qkvr.tile([128, N], BF, name=f"onorm{hh}")
                     for hh in range(2)]
            obounce = dram.tile([NCORES, 128, TOK], BF)
            orecvb = dram.tile([NCORES, 128, TOK], BF)
            orecv = x2p.tile([128, 8 * TOK], BF)
            with tc.tile_pool(name="sps", bufs=3, space="PSUM") as sps, \
                 tc.tile_pool(name="ops", bufs=4, space="PSUM") as ops, \
                 tc.tile_pool(name="ptp", bufs=6) as ptp, \
                 tc.tile_pool(name="nrm", bufs=3) as nrm:
                for c in range(4):
                    items = sched[c]
                    q0 = 512 * c
                    for h in range(2):
                        o_ps = ops.tile([65, 512], FP, tag="ops")
                        for idx, (kt, c0, c1, mk) in enumerate(items):
                            w = c1 - c0
                            s_ps = sps.tile([128, w], FP, tag="sps")
                            nc.tensor.matmul(
                                s_ps,
                                kT[64 * h:64 * h + 64,
                                   128 * kt:128 * kt + 128],
                                qT[64 * h:64 * h + 64, q0 + c0:q0 + c1],
                                start=True, stop=True)
                            p_sb = ptp.tile([128, w], BF, tag="pt")
                            nc.scalar.activation(out=p_sb, in_=s_ps,
                                                 func=AF.Exp, scale=0.125)
                            if mk is not None:
                                mo = MASK_OFF[mk]
                                nc.gpsimd.tensor_mul(
                                    p_sb[:, 0:128], p_sb[:, 0:128],
                                    mask_sb[:, mo:mo + 128])
                            nc.tensor.matmul(
                                o_ps[:, c0:c1],
                                vaug[kt][:, 65 * h:65 * h + 65],
                                p_sb, start=(idx == 0),
                                stop=(idx == len(items) - 1),
                                skip_group_check=True)
                        recip = nrm.tile([1, 512], FP, tag="recip")
                        nc.vector.reciprocal(out=recip, in_=o_ps[64:65, :])
                        rbc = nrm.tile([64, 512], FP, tag="rbc")
                        nc.gpsimd.partition_broadcast(rbc, recip)
                        nc.vector.tensor_mul(
                            onorm[c // 2][64 * h:64 * h + 64,
                                          q0 % N:q0 % N + 512],
                            o_ps[0:64, :], rbc)
                    # after both q-chunks of a half are done, stage that half
                    if c % 2 == 1:
                        hh = c // 2
                        nc.sync.dma_start(
                            out=obounce[4 * hh:4 * hh + 4].rearrange(
                                "j p t -> p j t"),
                            in_=onorm[hh].rearrange("p (j t) -> p j t", t=TOK))

            if single:
                nc.sync.dma_start(out=orecvb[:], in_=obounce[:])
            else:
                nc.gpsimd.collective_compute(
                    "AllToAll", ALU.bypass,
                    replica_groups=[list(range(NCORES))],
                    ins=[obounce.opt()], outs=[orecvb.opt()])
            nc.sync.dma_start(
                out=orecv.rearrange("p (j t) -> p j t", t=TOK),
                in_=orecvb.rearrange("j p t -> p j t"))

            # ---------------- phase 4: attn_out + residual -------------
            x2T = [x2p.tile([128, TOK], FP, name=f"x2T{m}") for m in range(8)]
            x2b = [x2p.tile([128, TOK], BF, name=f"x2b{m}") for m in range(8)]
            sqb = [x2p.tile([128, TOK], BF, name=f"sqb{m}") for m in range(8)]
            with tc.tile_pool(name="aops", bufs=2, space="PSUM") as aops:
                for g in range(2):
                    for mi in range(4):
                        m = 4 * g + mi
                        ps = aops.tile([128, TOK], FP, tag="aops")
                        for k in range(8):
                            off = 1024 * mi + 128 * k
                            nc.tensor.matmul(
                                ps, wao_sb[g][:, off:off + 128],
                                orecv[:, TOK * k:TOK * k + TOK],
                                start=(k == 0), stop=(k == 7))
                        nc.vector.scalar_tensor_tensor(
                            out=x2T[m], in0=ps, scalar=gmsa_sb[:, m:m + 1],
                            in1=xsT[m], op0=ALU.mult, op1=ALU.add)
                        nc.gpsimd.tensor_copy(out=x2b[m], in_=x2T[m])
                        nc.gpsimd.tensor_mul(sqb[m], x2b[m], x2b[m])

            # ---------------- phase 5: LN2 (gamma/beta folded on host) -
            h2T = [x2p.tile([128, TOK], BF, name=f"h2T{k}") for k in range(8)]
            with tc.tile_pool(name="l2ps", bufs=1, space="PSUM") as l2ps, \
                 tc.tile_pool(name="l2t", bufs=1) as l2t:
                sum_ps = l2ps.tile([1, TOK], FP, tag="l2sum")
                for k in range(8):
                    nc.tensor.matmul(sum_ps, ones_sb, x2b[k],
                                     start=(k == 0), stop=(k == 7))
                ssq_ps = l2ps.tile([1, TOK], FP, tag="l2ssq")
                for k in range(8):
                    nc.tensor.matmul(ssq_ps, ones_sb, sqb[k],
                                     start=(k == 0), stop=(k == 7),
                                     skip_group_check=True)
                mu2 = l2t.tile([1, TOK], BF)
                nc.vector.tensor_scalar_mul(mu2, sum_ps, 1.0 / D)
                mu2f = l2t.tile([1, TOK], FP)
                nc.vector.tensor_scalar_mul(mu2f, sum_ps, 1.0 / D)
                var2 = l2t.tile([1, TOK], FP)
                musq = l2t.tile([1, TOK], FP)
                nc.vector.tensor_mul(musq, mu2f, mu2f)
                nc.vector.tensor_scalar_mul(var2, ssq_ps, 1.0 / D)
                nc.vector.tensor_sub(var2, var2, musq)
                sd2 = l2t.tile([1, TOK], FP)
                nc.scalar.activation(out=sd2, in_=var2, func=AF.Sqrt,
                                     bias=eps1, scale=1.0)
                rstd2 = l2t.tile([1, TOK], BF)
                with nc.allow_low_precision(reason="rstd2 row as bf16 "
                                            "matmul-broadcast operand"):
                    nc.vector.reciprocal(out=rstd2, in_=sd2)
                # row broadcasts via PE rank-1
                mu2bc = l2ps.tile([128, TOK], FP, tag="l2mub")
                nc.tensor.matmul(mu2bc, ones_row, mu2,
                                 start=True, stop=True,
                                 skip_group_check=True)
                rstd2bc = l2ps.tile([128, TOK], FP, tag="l2rsb")
                nc.tensor.matmul(rstd2bc, ones_row, rstd2,
                                 start=True, stop=True,
                                 skip_group_check=True)
                for k in range(8):
                    u = l2t.tile([128, TOK], FP, tag="u", bufs=2)
                    nc.vector.tensor_sub(u, x2T[k], mu2bc)
                    nc.vector.tensor_mul(h2T[k], u, rstd2bc)

            # ---------------- phase 6: MLP -----------------------------
            g_sb = [gp.tile([128, TOK], BF, name=f"g{m}") for m in range(32)]
            with tc.tile_pool(name="m1ps", bufs=3, space="PSUM") as m1ps:
                for g in range(8):
                    for mi in range(4):
                        m = 4 * g + mi
                        ps = m1ps.tile([128, TOK], FP, tag="m1")
                        for k in range(8):
                            off = 1024 * mi + 128 * k
                            nc.tensor.matmul(ps, w1_sb[g][:, off:off + 128],
                                             h2T[k],
                                             start=(k == 0), stop=(k == 7))
                        gfunc = (AF.Identity if os.environ.get("DBG_NO_GELU")
                                 else AF.Gelu_apprx_tanh)
                        nc.scalar.activation(out=g_sb[m], in_=ps,
                                             func=gfunc,
                                             bias=b1_sb[:, m:m + 1],
                                             scale=1.0)

            with tc.tile_pool(name="m2ps", bufs=3, space="PSUM") as m2ps, \
                 tc.tile_pool(name="outp", bufs=3) as outp:
                for m in range(8):
                    ps = m2ps.tile([128, TOK], FP, tag="m2")
                    for k in range(32):
                        nc.tensor.matmul(ps, w2_sb[m][:, 128 * k:128 * k + 128],
                                         g_sb[k],
                                         start=(k == 0), stop=(k == 31))
                    outT = outp.tile([128, TOK], FP, tag="outT")
                    nc.vector.tensor_scalar(
                        out=outT, in0=ps, scalar1=b2_sb[:, m:m + 1],
                        scalar2=gmlp_sb[:, m:m + 1],
                        op0=ALU.add, op1=ALU.mult)
                    nc.vector.tensor_add(outT, outT, x2T[m])
                    nc.sync.dma_start(out=out_d[m], in_=outT)

            w2p.release()
            w1p.release()
            waop.release()

    nc.compile()
    return nc


# ---------------------------------------------------------------------------
# host side
# ---------------------------------------------------------------------------

_NC = None


def _get_nc():
    global _NC
    if _NC is None:
        _NC = build_program()
    return _NC


def _mask01_tiles():
    """[128,128] multiplicative 0/1 masks in S^T orientation (rows=k,
    cols=q), concatenated [diag | strict | incl]."""
    a = np.arange(128) // BS
    diag = (a[:, None] == a[None, :])
    strict = (a[None, :] > a[:, None])
    incl = (a[None, :] >= a[:, None])
    m = np.concatenate([diag, strict, incl], axis=1).astype(np.float32)
    return np.ascontiguousarray(m.astype(bf16))


def _tile4(wT, km, mm):
    """[K, M] -> (m, p, k, c) with arr[m, p, k, c] = wT[128k+p, 128m+c]."""
    return wT.reshape(km, 128, mm, 128).transpose(2, 1, 0, 3)


def _group(w4, gs):
    """(m, p, k, c) -> (g, p, m_in_g, k, c) groups of gs m-tiles."""
    mm, p, km, c = w4.shape
    return np.ascontiguousarray(
        w4.reshape(mm // gs, gs, p, km, c).transpose(0, 2, 1, 3, 4)
        .astype(bf16))


def _prep_inputs(x, c, cos, sin, norm1_w, qkv_w, attn_out_w, norm2_w,
                 mlp_w1, mlp_b1, mlp_w2, mlp_b2, adaLN_w, adaLN_b):
    f32 = np.float32
    x = np.asarray(x, f32).reshape(S, D)
    c = np.asarray(c, f32).reshape(COND)
    cos = np.asarray(cos, f32)
    sin = np.asarray(sin, f32)
    qkv_w = np.asarray(qkv_w, f32)
    mlp_w1 = np.asarray(mlp_w1, f32)

    # adaLN modulation on host
    mods = adaLN_w.astype(f32) @ c + np.asarray(adaLN_b, f32)
    sh_msa, sc_msa, g_msa, sh_mlp, sc_mlp, g_mlp = mods.reshape(6, D)

    gam1 = (1.0 + sc_msa) * np.asarray(norm1_w, f32)          # [D]
    qkv_ws = qkv_w * gam1[None, :]                            # [3D, D]
    u_qkv = qkv_ws.sum(axis=1)                                # [3D]
    b_qkv = qkv_w @ sh_msa                                    # [3D]

    gam2 = (1.0 + sc_mlp) * np.asarray(norm2_w, f32)          # [D]
    w1s = mlp_w1 * gam2[None, :]                              # [4D, D]
    b1f = np.asarray(mlp_b1, f32) + mlp_w1 @ sh_mlp           # [4D]

    xb = x.astype(bf16)
    xT = np.ascontiguousarray(
        xb.T.reshape(8, 128, S))                              # (k, p, t)

    # rope tables expanded to S columns, 1/sd folded on device
    cs = np.concatenate([cos, cos], axis=-1).T                # [64, N]
    ss = np.concatenate([sin.T, -sin.T], axis=0)              # [64, N] swapped
    cos2 = np.tile(np.vstack([cs, cs]), (1, 2))               # [128, S]
    sin2 = np.tile(np.vstack([ss, ss]), (1, 2))               # [128, S]
    trig = np.ascontiguousarray(np.hstack([cos2, sin2]).astype(bf16))

    waoT = _group(_tile4(np.asarray(attn_out_w, f32).T, 8, 8), 4)
    w1T = _group(_tile4(w1s.T, 8, 32), 4)
    w2T = np.ascontiguousarray(
        _tile4(np.asarray(mlp_w2, f32).T, 32, 8).astype(bf16))

    smallc = np.ascontiguousarray(np.hstack([
        g_msa.reshape(8, 128).T,
        g_mlp.reshape(8, 128).T,
        b1f.reshape(32, 128).T,
        np.asarray(mlp_b2, f32).reshape(8, 128).T]).astype(f32))  # [128, 56]

    common = {
        "x": np.ascontiguousarray(xb),
        "xT": xT,
        "waoT": waoT, "w1T": w1T, "w2T": w2T,
        "smallc": smallc, "trig": trig,
        "mask01": _mask01_tiles(),
    }
    in_maps = []
    for j in range(NCORES):
        wq = np.stack([
            np.ascontiguousarray(
                qkv_ws[s * D + 128 * j: s * D + 128 * j + 128].T
                .reshape(8, 128, 128))
            for s in range(3)])  # [3, k, p, c]
        wq = np.ascontiguousarray(wq.transpose(0, 2, 1, 3).astype(bf16))
        ub = np.stack([
            np.concatenate([u_qkv[s * D + 128 * j: s * D + 128 * j + 128]
                            for s in range(3)]),
            np.concatenate([b_qkv[s * D + 128 * j: s * D + 128 * j + 128]
                            for s in range(3)])])  # [2, 384]
        m = dict(common)
        m["wqkvT"] = wq  # [3, 128, 8, 128] = (s, p, k, c)
        m["ubrow"] = np.ascontiguousarray(ub.astype(bf16))
        m["xsliceT"] = np.ascontiguousarray(
            xT[:, :, TOK * j:TOK * j + TOK])
        in_maps.append(m)
    return in_maps


def _assemble(res):
    """Gather per-core feature-major outputs into the full [1, S, D]."""
    parts = []
    for j in range(NCORES):
        o = res.results[j]["out"]  # [8, 128, TOK] feature-major
        parts.append(np.ascontiguousarray(
            o.transpose(2, 0, 1).reshape(TOK, D)))
    return np.concatenate(parts, axis=0).reshape(1, S, D).astype(np.float32)


def kernel(**inputs):
    nc = _get_nc()
    in_maps = _prep_inputs(**inputs)
    res = run_bass_kernel_spmd(nc, in_maps, core_ids=list(range(NCORES)))
    return _assemble(res)


# revision 31
# speedup vs baseline: 1.2736x; 1.2736x over previous
"""DiT block with block-diffusion sparse attention on 8 Trainium2 NeuronCores.

v3 strategy:
  - adaLN modulation computed on HOST (tiny matvec); gamma scales folded into
    the QKV / MLP1 weights, shifts folded into bias rows.
  - LN1 never materializes h: QKV matmuls read host-pretransposed x^T
    directly; the per-token (-mu, sigma) correction enters the same PSUM as a
    rank-2 matmul with stationary (u, b) = ((W*gamma)@1, W@beta); the 1/sigma
    scale is folded into the RoPE cos/sin tables. Stats come from a
    token-major bn_stats pass.
  - Attention: 2 heads per core, scores in S^T orientation, both heads of an
    item share one PSUM tile / one exp / one mask op. Softmax normalization
    is DEFERRED: unnormalized o + denominator rows are evacuated per stream,
    reciprocals batched per half (keeps gpsimd/vector off the per-stream
    critical path).
  - PSUM is always evacuated through ScalarE (ACT) before VectorE touches the
    data - DVE reads from PSUM are ~4x slower than from SBUF.
  - One AllToAll converts head-sharded attention output to token-sharded.
  - attn_out, LN2, MLP token-sharded with full bf16 weights prefetched during
    the attention/A2A window. Output written feature-major; host transposes.
"""

import os
import numpy as np
import ml_dtypes

import concourse.bass as bass
import concourse.tile as tile
from concourse import bacc, mybir
from concourse.bass_utils import run_bass_kernel_spmd
from concourse.masks import make_identity

bf16 = ml_dtypes.bfloat16
FP = mybir.dt.float32
BF = mybir.dt.bfloat16
AF = mybir.ActivationFunctionType
ALU = mybir.AluOpType

NCORES = 8
S, N, D, H, HD, BS, COND = 2048, 1024, 1024, 16, 64, 16, 128
TOK = S // NCORES  # 256 tokens per core after A2A


def _attn_schedule():
    """Per q-chunk list of (ktile, col0, col1, mask) in S^T orientation."""
    sched = []
    for c in range(4):
        items = []
        if c < 2:  # noisy q chunk
            for j in range(4 * c + 4):  # clean k tiles, bq > bk
                js = j - 4 * c
                if js < 0:
                    items.append((8 + j, 0, 512, None))
                else:
                    items.append((8 + j, 128 * js, 512, "strict"))
            for s in range(4):  # own-block diagonal (noisy k)
                items.append((4 * c + s, 128 * s, 128 * s + 128, "diag"))
        else:  # clean q chunk, bq >= bk
            cq = c - 2
            for j in range(4 * cq + 4):
                js = j - 4 * cq
                if js < 0:
                    items.append((8 + j, 0, 512, None))
                else:
                    items.append((8 + j, 128 * js, 512, "incl"))
        assert items[0][1] == 0 and items[0][2] == 512
        sched.append(items)
    return sched


MASK_OFF = {"diag": 0, "strict": 256, "incl": 512}


def build_program(single=False, dbg=False):
    """single=True builds a 1-device variant (A2A replaced by a local DMA
    copy) for TimelineSim cost-model analysis. dbg=True adds a debug output
    with intermediate tensors."""
    nc = bacc.Bacc(
        "TRN2", target_bir_lowering=False, debug=False,
        enable_asserts=False, num_devices=1 if single else NCORES,
    )

    def din(name, shape, dt=BF):
        return nc.dram_tensor(name, shape, dt, kind="ExternalInput").ap()

    x_d = din("x", [S, D])                            # token-major (stats)
    xT_d = din("xT", [8, 128, S])                     # feature-major (k, p, t)
    xsT_d = din("xsliceT", [8, 128, TOK])             # residual slice (k, p, t)
    trig_d = din("trig", [128, 2 * S])                # cos2 | sin2(dest-signed)
    mask01_d = din("mask01", [128, 768])              # diag|strict|incl x2
    wqkv_d = din("wqkvT", [3, 128, 8, 128])           # (s, p, k, c) g-scaled
    ub_d = din("ubrow", [2, 384])                     # (u; b) per-core slice
    wao_d = din("waoT", [2, 128, 4, 8, 128])          # (g, p, mi, k, c)
    w1_d = din("w1T", [8, 128, 4, 8, 128])            # (g, p, mi, k, c) scaled
    w2_d = din("w2T", [8, 128, 32, 128])              # (m, p, k, c)
    smallc_d = din("smallc", [128, 64], FP)           # gmsa|gmlp|b1'|b2|gb2
    out_d = nc.dram_tensor("out", [8, 128, TOK], FP, kind="ExternalOutput").ap()
    dbg_d = (nc.dram_tensor("dbg", [8, 128, S], BF,
                            kind="ExternalOutput").ap() if dbg else None)

    sched = _attn_schedule()

    with tile.TileContext(nc) as tc:
        with tc.tile_pool(name="const", bufs=1) as const, \
             tc.tile_pool(name="dram", bufs=1, space="DRAM") as dram, \
             tc.tile_pool(name="qkvr", bufs=1) as qkvr, \
             tc.tile_pool(name="vaugp", bufs=1) as vaugp, \
             tc.tile_pool(name="x2p", bufs=1) as x2p, \
             tc.tile_pool(name="gp", bufs=1) as gp:

            # ---------------- early DMAs (stats x first, then xT) ------
            xstat = tc.alloc_tile_pool(name="xstat", bufs=2)
            xTp = tc.alloc_tile_pool(name="xTp", bufs=1)
            x_r = x_d.rearrange("(t p) d -> p t d", p=128)  # [128,16,D]
            xg_sb = []
            for g in range(4):
                xg = xstat.tile([128, 4, D], BF, tag="x", name=f"xg{g}")
                nc.sync.dma_start(out=xg, in_=x_r[:, 4 * g:4 * g + 4, :])
                xg_sb.append(xg)
            xT_sb = [xTp.tile([128, S], BF, name=f"xT{k}") for k in range(8)]
            for k in range(8):
                nc.sync.dma_start(out=xT_sb[k], in_=xT_d[k])

            # ---------------- constants / small inputs ----------------
            trig_sb = const.tile([128, 2 * S], BF)
            nc.sync.dma_start(out=trig_sb, in_=trig_d)
            mask_sb = const.tile([128, 768], BF)
            nc.sync.dma_start(out=mask_sb, in_=mask01_d)
            smallc = const.tile([128, 64], FP)
            nc.sync.dma_start(out=smallc, in_=smallc_d)
            gmsa_sb = smallc[:, 0:8]
            gmlp_sb = smallc[:, 8:16]
            b1_sb = smallc[:, 16:48]
            b2_sb = smallc[:, 48:56]
            gb2_sb = smallc[:, 56:64]
            ub_sb = const.tile([2, 384], BF)
            nc.scalar.dma_start(out=ub_sb, in_=ub_d)
            ones_sb = const.tile([128, 1], BF)
            nc.vector.memset(ones_sb, 1.0)
            ones_row = const.tile([1, 128], BF)
            nc.vector.memset(ones_row, 1.0)
            eps128 = const.tile([128, 1], FP)
            nc.vector.memset(eps128, 1e-5)
            eps1 = const.tile([1, 1], FP)
            nc.vector.memset(eps1, 1e-5)
            ident_f = const.tile([128, 128], FP)
            make_identity(nc, ident_f)
            ident_b = const.tile([128, 128], BF)
            nc.vector.tensor_copy(out=ident_b, in_=ident_f)

            # residual slice (feature-major) straight from DRAM
            xsT = [x2p.tile([128, TOK], BF, name=f"xsT{j}") for j in range(8)]
            for fj in range(8):
                nc.sync.dma_start(out=xsT[fj], in_=xsT_d[fj])

            # qkv weights early on the scalar queue
            wqkvp = tc.alloc_tile_pool(name="wqkvp", bufs=1)
            wq_sb = [wqkvp.tile([128, 8 * 128], BF, name=f"wq{m}")
                     for m in range(3)]
            for m in range(3):
                nc.scalar.dma_start(
                    out=wq_sb[m].rearrange("p (k c) -> p k c", c=128),
                    in_=wqkv_d[m])

            # ---------------- phase 0: LN1 stats (token-major) ---------
            # per 128-token tile: bn stats -> (-mu, sd, rstd) columns of a
            # [128, 48] tile; one PE transpose + DRAM bounce turns them into
            # rows aligned with qT columns.
            stats_dr = dram.tile([3, S], BF)
            rows_sb = const.tile([2, S], BF)   # (negmu ; sd)
            rstd_row = const.tile([1, S], BF)
            with tc.tile_pool(name="statp", bufs=4) as statp, \
                 tc.tile_pool(name="st48", bufs=1) as st48p, \
                 tc.tile_pool(name="stps", bufs=1, space="PSUM") as stps:
                stat48 = st48p.tile([128, 48], FP)
                for g in range(4):
                    for sub in range(4):
                        ti = 4 * g + sub
                        x_sb = xg_sb[g][:, sub, :]
                        st = statp.tile([128, 2, 6], FP, tag="bst")
                        for sg in range(2):
                            nc.vector.bn_stats(
                                out=st[:, sg, :],
                                in_=x_sb[:, 512 * sg:512 * sg + 512])
                        mv = statp.tile([128, 2], FP, tag="mv")
                        nc.vector.bn_aggr(out=mv, in_=st)
                        nc.vector.tensor_scalar_mul(
                            stat48[:, ti:ti + 1], mv[:, 0:1], -1.0)
                        nc.scalar.activation(
                            out=stat48[:, 16 + ti:17 + ti], in_=mv[:, 1:2],
                            func=AF.Sqrt, bias=eps128, scale=1.0)
                        nc.vector.reciprocal(
                            out=stat48[:, 32 + ti:33 + ti],
                            in_=stat48[:, 16 + ti:17 + ti])
                ps = stps.tile([48, 128], FP)
                nc.tensor.transpose(ps, stat48, ident_f)
                st48b = st48p.tile([48, 128], BF)
                nc.vector.tensor_copy(out=st48b, in_=ps)
                nc.sync.dma_start(
                    out=stats_dr.rearrange("v (t p) -> (v t) p", p=128),
                    in_=st48b)
                nc.sync.dma_start(out=rows_sb, in_=stats_dr[0:2])
                nc.sync.dma_start(out=rstd_row, in_=stats_dr[2:3])

            # rstd broadcast via PE rank-1 -> ACT evac -> fold into trig
            cosr = const.tile([128, S], BF)
            sinr = const.tile([128, S], BF)
            with tc.tile_pool(name="rbps", bufs=1, space="PSUM") as rbps, \
                 tc.tile_pool(name="rbt", bufs=1) as rbt:
                rstd_ps = rbps.tile([128, S], FP)
                for q in range(4):
                    nc.tensor.matmul(
                        rstd_ps[:, 512 * q:512 * q + 512], ones_row,
                        rstd_row[:, 512 * q:512 * q + 512],
                        start=True, stop=True, skip_group_check=True)
                rstd_bc = rbt.tile([128, S], BF)
                for q in range(2):
                    qs = slice(1024 * q, 1024 * q + 1024)
                    nc.scalar.copy(out=rstd_bc[:, qs], in_=rstd_ps[:, qs])
                nc.vector.tensor_mul(cosr, trig_sb[:, 0:S], rstd_bc)
                nc.vector.tensor_mul(sinr, trig_sb[:, S:2 * S], rstd_bc)

            # ---------------- phase 1: QKV + RoPE ----------------------
            qT = qkvr.tile([128, S], BF)
            kT = qkvr.tile([128, S], BF)
            vT = qkvr.tile([128, S], BF)
            qkv_dst = [qT, kT, vT]

            with tc.tile_pool(name="ropep", bufs=3) as ropep, \
                 tc.tile_pool(name="qkvps", bufs=3, space="PSUM") as qkvps:
                for n in range(4):
                    nsl = slice(512 * n, 512 * n + 512)
                    for m in range(3):
                        ps = qkvps.tile([128, 512], FP, tag="qkvps")
                        for k in range(8):
                            nc.tensor.matmul(
                                ps, wq_sb[m][:, 128 * k:128 * k + 128],
                                xT_sb[k][:, nsl],
                                start=(k == 0), stop=False)
                        # rank-2: + u (.) (-mu)  +  b (.) sd
                        nc.tensor.matmul(
                            ps, ub_sb[:, 128 * m:128 * m + 128],
                            rows_sb[0:2, nsl], start=False, stop=True,
                            skip_group_check=True)
                        # ACT evac, then rope on SBUF bf16:
                        # dst = pb*cosr + swap32(pb)*sinr  (sign in sinr)
                        pb = ropep.tile([128, 512], BF, tag="pb")
                        nc.scalar.copy(out=pb, in_=ps)
                        pbs = ropep.tile([128, 512], BF, tag="pbs")
                        for h in range(2):
                            r = 64 * h
                            nc.vector.tensor_copy(
                                out=pbs[r:r + 32, :], in_=pb[r + 32:r + 64, :])
                            nc.vector.tensor_copy(
                                out=pbs[r + 32:r + 64, :], in_=pb[r:r + 32, :])
                        t1 = ropep.tile([128, 512], BF, tag="t1")
                        nc.vector.tensor_mul(t1, pb, cosr[:, nsl])
                        t2 = ropep.tile([128, 512], BF, tag="t2")
                        nc.vector.tensor_mul(t2, pbs, sinr[:, nsl])
                        nc.vector.tensor_add(
                            qkv_dst[m][:, nsl], t1, t2)
            wqkvp.release()
            xTp.release()
            xstat.release()

            # ---------------- phase 2: V token-major (+ones col) -------
            vaug = [vaugp.tile([128, 130], BF, name=f"vaug{kt}")
                    for kt in range(16)]
            with tc.tile_pool(name="vtps", bufs=2, space="PSUM") as vtps:
                for kt in range(16):
                    ps = vtps.tile([128, 128], BF, tag="vt")
                    nc.tensor.transpose(
                        ps, vT[:, 128 * kt:128 * kt + 128], ident_b)
                    va = vaug[kt]
                    nc.vector.memset(va[:, 64:65], 1.0)
                    nc.vector.memset(va[:, 129:130], 1.0)
                    nc.scalar.copy(
                        out=va[:, 0:130].rearrange(
                            "p (h y) -> p h y", y=65)[:, :, 0:64],
                        in_=ps.rearrange("p (h d) -> p h d", d=64))

            # ---------------- weight prefetch (runs under attn + A2A) --
            waop = tc.alloc_tile_pool(name="waop", bufs=2)
            w1p = tc.alloc_tile_pool(name="w1p", bufs=4)
            w2p = tc.alloc_tile_pool(name="w2p", bufs=3)
            wao_sb = [waop.tile([128, 4 * 8 * 128], BF, tag="wao",
                                name=f"wao{g}")
                      for g in range(2)]
            for g in range(2):
                nc.scalar.dma_start(
                    out=wao_sb[g].rearrange("p (mi k c) -> p mi k c",
                                            k=8, c=128),
                    in_=wao_d[g])
            w1_sb = [w1p.tile([128, 4 * 8 * 128], BF, tag="w1",
                              name=f"w1_{g}")
                     for g in range(8)]
            for g in range(8):
                nc.scalar.dma_start(
                    out=w1_sb[g].rearrange("p (mi k c) -> p mi k c",
                                           k=8, c=128),
                    in_=w1_d[g])
            w2_sb = [w2p.tile([128, 32 * 128], BF, tag="w2",
                              name=f"w2_{m}")
                     for m in range(8)]
            for m in range(8):
                nc.scalar.dma_start(
                    out=w2_sb[m].rearrange("p (k c) -> p k c", c=128),
                    in_=w2_d[m])

            # ---------------- phase 3: sparse attention ----------------
            # Both heads of an item share one [128, 2, 512] score PSUM tile,
            # one exp, one mask op. Normalization deferred: o_un + den rows
            # evacuated per stream; reciprocal batched per half.
            onorm = [qkvr.tile([128, N], BF, name=f"onorm{hh}")
                     for hh in range(2)]
            obounce = dram.tile([NCORES, 128, TOK], BF)
            orecvb = dram.tile([NCORES, 128, TOK], BF)
            orecv = x2p.tile([128, 8 * TOK], BF)
            o_un = [qkvr.tile([64, 512], BF, name=f"oun{k}")
                    for k in range(8)]
            # den rows live at partitions {0,32,64,96} (safe write offsets);
            # memset 1.0 so the reciprocal over unused partitions is benign
            den4 = [qkvr.tile([128, 512], FP, name=f"den{hh}")
                    for hh in range(2)]
            recip4 = [qkvr.tile([128, 512], BF, name=f"recip{hh}")
                      for hh in range(2)]
            for hh in range(2):
                nc.vector.memset(den4[hh], 1.0)
            with tc.tile_pool(name="sps", bufs=2, space="PSUM") as sps, \
                 tc.tile_pool(name="ops", bufs=4, space="PSUM") as ops, \
                 tc.tile_pool(name="ptp", bufs=4) as ptp, \
                 tc.tile_pool(name="nrm", bufs=2) as nrm:
                for c in range(4):
                    items = sched[c]
                    q0 = 512 * c
                    o_ps = [ops.tile([65, 512], FP, tag="ops",
                                     name=f"ops{c}_{h}") for h in range(2)]
                    for idx, (kt, c0, c1, mk) in enumerate(items):
                        w = c1 - c0
                        s_ps = sps.tile([128, 2, 512], FP, tag="sps")
                        for h in range(2):
                            nc.tensor.matmul(
                                s_ps[:, h, 0:w],
                                kT[64 * h:64 * h + 64,
                                   128 * kt:128 * kt + 128],
                                qT[64 * h:64 * h + 64, q0 + c0:q0 + c1],
                                start=True, stop=True, skip_group_check=True)
                        p_sb = ptp.tile([128, 2, 512], BF, tag="pt")
                        if w == 512:
                            nc.scalar.activation(out=p_sb[:, :, :],
                                                 in_=s_ps[:, :, :],
                                                 func=AF.Exp, scale=0.125)
                        else:
                            for h in range(2):
                                nc.scalar.activation(out=p_sb[:, h, 0:w],
                                                     in_=s_ps[:, h, 0:w],
                                                     func=AF.Exp, scale=0.125)
                        if mk is not None:
                            mo = MASK_OFF[mk]
                            for h in range(2):
                                nc.gpsimd.tensor_mul(
                                    p_sb[:, h, 0:128], p_sb[:, h, 0:128],
                                    mask_sb[:, mo + 128 * h:mo + 128 * h + 128])
                        for h in range(2):
                            nc.tensor.matmul(
                                o_ps[h][:, c0:c1],
                                vaug[kt][:, 65 * h:65 * h + 65],
                                p_sb[:, h, 0:w], start=(idx == 0),
                                stop=(idx == len(items) - 1),
                                skip_group_check=True)
                    for h in range(2):
                        k = 2 * c + h
                        r = 32 * (k % 4)
                        nc.scalar.copy(out=o_un[k], in_=o_ps[h][0:64, :])
                        nc.scalar.copy(out=den4[c // 2][r:r + 1, :],
                                       in_=o_ps[h][64:65, :])
                    # per-half deferred normalization + staging
                    if c % 2 == 1:
                        hh = c // 2
                        with nc.allow_low_precision(reason="softmax denom "
                                                    "recip as bf16"):
                            nc.vector.reciprocal(out=recip4[hh],
                                                 in_=den4[hh])
                        for kk in range(4):
                            k = 4 * hh + kk
                            cc, h = k // 2, k % 2
                            # partition_broadcast reads partition 0 of the
                            # TILE (AP partition offset ignored): stage row
                            rtmp = nrm.tile([1, 512], BF, tag="rtmp", bufs=2)
                            nc.scalar.copy(
                                out=rtmp, in_=recip4[hh][32 * kk:32 * kk + 1, :])
                            rbc = nrm.tile([64, 512], BF, tag="rbc", bufs=3)
                            nc.gpsimd.partition_broadcast(rbc, rtmp)
                            nc.vector.tensor_mul(
                                onorm[hh][64 * h:64 * h + 64,
                                          (512 * cc) % N:(512 * cc) % N + 512],
                                o_un[k], rbc)
                        nc.sync.dma_start(
                            out=obounce[4 * hh:4 * hh + 4].rearrange(
                                "j p t -> p j t"),
                            in_=onorm[hh].rearrange("p (j t) -> p j t", t=TOK))

            if dbg_d is not None:
                nc.sync.dma_start(out=dbg_d[0], in_=qT)
                nc.sync.dma_start(out=dbg_d[1], in_=kT)
                nc.sync.dma_start(out=dbg_d[2], in_=vT)
                nc.sync.dma_start(out=dbg_d[3][:, 0:N], in_=onorm[0])
                nc.sync.dma_start(out=dbg_d[4][:, 0:N], in_=onorm[1])
                nc.sync.dma_start(out=dbg_d[5][0:64, 0:512], in_=o_un[0])
                nc.sync.dma_start(out=dbg_d[5][0:64, 512:1024], in_=o_un[1])
                nc.sync.dma_start(out=dbg_d[5][0:64, 1024:1536], in_=o_un[6])
                nc.sync.dma_start(out=dbg_d[5][0:64, 1536:2048], in_=o_un[7])
                dbgrc = qkvr.tile([128, 1024], BF)
                nc.vector.tensor_copy(out=dbgrc[:, 0:512], in_=den4[0])
                nc.vector.tensor_copy(out=dbgrc[:, 512:1024], in_=den4[1])
                nc.sync.dma_start(out=dbg_d[6][:, 0:1024], in_=dbgrc)
                nc.sync.dma_start(out=dbg_d[6][:, 1024:1536], in_=recip4[0])
                nc.sync.dma_start(out=dbg_d[6][:, 1536:2048], in_=recip4[1])
            if single:
                nc.sync.dma_start(out=orecvb[:], in_=obounce[:])
            else:
                nc.gpsimd.collective_compute(
                    "AllToAll", ALU.bypass,
                    replica_groups=[list(range(NCORES))],
                    ins=[obounce.opt()], outs=[orecvb.opt()])
            nc.sync.dma_start(
                out=orecv.rearrange("p (j t) -> p j t", t=TOK),
                in_=orecvb.rearrange("j p t -> p j t"))
            if dbg_d is not None:
                nc.sync.dma_start(out=dbg_d[7], in_=orecv)

            # ---------------- phase 4: attn_out + residual -------------
            x2T = [x2p.tile([128, TOK], FP, name=f"x2T{m}") for m in range(8)]
            x2b = [x2p.tile([128, TOK], BF, name=f"x2b{m}") for m in range(8)]
            sqb = [x2p.tile([128, TOK], BF, name=f"sqb{m}") for m in range(8)]
            with tc.tile_pool(name="aops", bufs=3, space="PSUM") as aops, \
                 tc.tile_pool(name="aot", bufs=3) as aot:
                for g in range(2):
                    for mi in range(4):
                        m = 4 * g + mi
                        ps = aops.tile([128, TOK], FP, tag="aops")
                        for k in range(8):
                            off = 1024 * mi + 128 * k
                            nc.tensor.matmul(
                                ps, wao_sb[g][:, off:off + 128],
                                orecv[:, TOK * k:TOK * k + TOK],
                                start=(k == 0), stop=(k == 7))
                        ao_sb = aot.tile([128, TOK], FP, tag="ao")
                        nc.scalar.copy(out=ao_sb, in_=ps)
                        nc.vector.scalar_tensor_tensor(
                            out=x2T[m], in0=ao_sb,
                            scalar=gmsa_sb[:, m:m + 1],
                            in1=xsT[m], op0=ALU.mult, op1=ALU.add)
                        nc.vector.tensor_copy(out=x2b[m], in_=x2T[m])
                        nc.vector.tensor_mul(sqb[m], x2b[m], x2b[m])

            # ---------------- phase 5: LN2 (gamma/beta folded on host) -
            h2T = [x2p.tile([128, TOK], BF, name=f"h2T{k}") for k in range(8)]
            with tc.tile_pool(name="l2ps", bufs=1, space="PSUM") as l2ps, \
                 tc.tile_pool(name="l2t", bufs=1) as l2t:
                sum_ps = l2ps.tile([1, TOK], FP, tag="l2sum")
                for k in range(8):
                    nc.tensor.matmul(sum_ps, ones_sb, x2b[k],
                                     start=(k == 0), stop=(k == 7))
                ssq_ps = l2ps.tile([1, TOK], FP, tag="l2ssq")
                for k in range(8):
                    nc.tensor.matmul(ssq_ps, ones_sb, sqb[k],
                                     start=(k == 0), stop=(k == 7),
                                     skip_group_check=True)
                mu2 = l2t.tile([1, TOK], BF)
                nc.vector.tensor_scalar_mul(mu2, sum_ps, 1.0 / D)
                mu2f = l2t.tile([1, TOK], FP)
                nc.vector.tensor_scalar_mul(mu2f, sum_ps, 1.0 / D)
                var2 = l2t.tile([1, TOK], FP)
                musq = l2t.tile([1, TOK], FP)
                nc.vector.tensor_mul(musq, mu2f, mu2f)
                nc.vector.tensor_scalar_mul(var2, ssq_ps, 1.0 / D)
                nc.vector.tensor_sub(var2, var2, musq)
                sd2 = l2t.tile([1, TOK], FP)
                nc.scalar.activation(out=sd2, in_=var2, func=AF.Sqrt,
                                     bias=eps1, scale=1.0)
                rstd2 = l2t.tile([1, TOK], BF)
                with nc.allow_low_precision(reason="rstd2 row as bf16 "
                                            "matmul-broadcast operand"):
                    nc.vector.reciprocal(out=rstd2, in_=sd2)
                # row broadcasts via PE rank-1 + ACT evac
                mu2bc_ps = l2ps.tile([128, TOK], FP, tag="l2mub")
                nc.tensor.matmul(mu2bc_ps, ones_row, mu2,
                                 start=True, stop=True,
                                 skip_group_check=True)
                rstd2bc_ps = l2ps.tile([128, TOK], FP, tag="l2rsb")
                nc.tensor.matmul(rstd2bc_ps, ones_row, rstd2,
                                 start=True, stop=True,
                                 skip_group_check=True)
                mu2bc = l2t.tile([128, TOK], FP)
                nc.scalar.copy(out=mu2bc, in_=mu2bc_ps)
                rstd2bc = l2t.tile([128, TOK], FP)
                nc.scalar.copy(out=rstd2bc, in_=rstd2bc_ps)
                for k in range(8):
                    u = l2t.tile([128, TOK], FP, tag="u", bufs=2)
                    nc.vector.tensor_sub(u, x2T[k], mu2bc)
                    nc.vector.tensor_mul(h2T[k], u, rstd2bc)

            # ---------------- phase 6: MLP -----------------------------
            g_sb = [gp.tile([128, TOK], BF, name=f"g{m}") for m in range(32)]
            with tc.tile_pool(name="m1ps", bufs=3, space="PSUM") as m1ps:
                for g in range(8):
                    for mi in range(4):
                        m = 4 * g + mi
                        ps = m1ps.tile([128, TOK], FP, tag="m1")
                        for k in range(8):
                            off = 1024 * mi + 128 * k
                            nc.tensor.matmul(ps, w1_sb[g][:, off:off + 128],
                                             h2T[k],
                                             start=(k == 0), stop=(k == 7))
                        gfunc = (AF.Identity if os.environ.get("DBG_NO_GELU")
                                 else AF.Gelu_apprx_tanh)
                        nc.scalar.activation(out=g_sb[m], in_=ps,
                                             func=gfunc,
                                             bias=b1_sb[:, m:m + 1],
                                             scale=1.0)

            with tc.tile_pool(name="m2ps", bufs=3, space="PSUM") as m2ps, \
                 tc.tile_pool(name="outp", bufs=3) as outp:
                for m in range(8):
                    ps = m2ps.tile([128, TOK], FP, tag="m2")
                    for k in range(32):
                        nc.tensor.matmul(ps, w2_sb[m][:, 128 * k:128 * k + 128],
                                         g_sb[k],
                                         start=(k == 0), stop=(k == 31))
                    # (ps + b2)*gmlp = gmlp*ps + gmlp*b2; gb2 precomputed
                    mo = outp.tile([128, TOK], FP, tag="mo")
                    nc.scalar.activation(out=mo, in_=ps, func=AF.Identity,
                                         bias=gb2_sb[:, m:m + 1],
                                         scale=gmlp_sb[:, m:m + 1])
                    outT = outp.tile([128, TOK], FP, tag="outT")
                    nc.vector.tensor_add(outT, mo, x2T[m])
                    nc.sync.dma_start(out=out_d[m], in_=outT)

            w2p.release()
            w1p.release()
            waop.release()

    nc.compile()
    return nc


# ---------------------------------------------------------------------------
# host side
# ---------------------------------------------------------------------------

_NC = None


def _get_nc():
    global _NC
    if _NC is None:
        _NC = build_program()
    return _NC


def _mask01_tiles():
    """[128,128] multiplicative 0/1 masks in S^T orientation (rows=k,
    cols=q), each doubled for the 2-head layout: [diag x2 | strict x2 |
    incl x2]."""
    a = np.arange(128) // BS
    diag = (a[:, None] == a[None, :])
    strict = (a[None, :] > a[:, None])
    incl = (a[None, :] >= a[:, None])
    m = np.concatenate([diag, diag, strict, strict, incl, incl],
                       axis=1).astype(np.float32)
    return np.ascontiguousarray(m.astype(bf16))


def _tile4(wT, km, mm):
    """[K, M] -> (m, p, k, c) with arr[m, p, k, c] = wT[128k+p, 128m+c]."""
    return wT.reshape(km, 128, mm, 128).transpose(2, 1, 0, 3)


def _group(w4, gs):
    """(m, p, k, c) -> (g, p, m_in_g, k, c) groups of gs m-tiles."""
    mm, p, km, c = w4.shape
    return np.ascontiguousarray(
        w4.reshape(mm // gs, gs, p, km, c).transpose(0, 2, 1, 3, 4)
        .astype(bf16))


def _prep_inputs(x, c, cos, sin, norm1_w, qkv_w, attn_out_w, norm2_w,
                 mlp_w1, mlp_b1, mlp_w2, mlp_b2, adaLN_w, adaLN_b):
    f32 = np.float32
    x = np.asarray(x, f32).reshape(S, D)
    c = np.asarray(c, f32).reshape(COND)
    cos = np.asarray(cos, f32)
    sin = np.asarray(sin, f32)
    qkv_w = np.asarray(qkv_w, f32)
    mlp_w1 = np.asarray(mlp_w1, f32)

    # adaLN modulation on host
    mods = adaLN_w.astype(f32) @ c + np.asarray(adaLN_b, f32)
    sh_msa, sc_msa, g_msa, sh_mlp, sc_mlp, g_mlp = mods.reshape(6, D)

    gam1 = (1.0 + sc_msa) * np.asarray(norm1_w, f32)          # [D]
    qkv_ws = qkv_w * gam1[None, :]                            # [3D, D]
    u_qkv = qkv_ws.sum(axis=1)                                # [3D]
    b_qkv = qkv_w @ sh_msa                                    # [3D]

    gam2 = (1.0 + sc_mlp) * np.asarray(norm2_w, f32)          # [D]
    w1s = mlp_w1 * gam2[None, :]                              # [4D, D]
    b1f = np.asarray(mlp_b1, f32) + mlp_w1 @ sh_mlp           # [4D]
    b2 = np.asarray(mlp_b2, f32)

    xb = x.astype(bf16)
    xT = np.ascontiguousarray(
        xb.T.reshape(8, 128, S))                              # (k, p, t)

    # rope tables expanded to S columns; sin table is DEST-signed for the
    # pure-swap pbs layout: rows 0:32 get -sin (they receive p[32:64]),
    # rows 32:64 get +sin.
    cs = np.concatenate([cos, cos], axis=-1).T                # [64, N]
    ss = np.concatenate([-sin.T, sin.T], axis=0)              # [64, N]
    cos2 = np.tile(np.vstack([cs, cs]), (1, 2))               # [128, S]
    sin2 = np.tile(np.vstack([ss, ss]), (1, 2))               # [128, S]
    trig = np.ascontiguousarray(np.hstack([cos2, sin2]).astype(bf16))

    waoT = _group(_tile4(np.asarray(attn_out_w, f32).T, 8, 8), 4)
    w1T = _group(_tile4(w1s.T, 8, 32), 4)
    w2T = np.ascontiguousarray(
        _tile4(np.asarray(mlp_w2, f32).T, 32, 8).astype(bf16))

    smallc = np.ascontiguousarray(np.hstack([
        g_msa.reshape(8, 128).T,
        g_mlp.reshape(8, 128).T,
        b1f.reshape(32, 128).T,
        b2.reshape(8, 128).T,
        (g_mlp * b2).reshape(8, 128).T]).astype(f32))         # [128, 64]

    common = {
        "x": np.ascontiguousarray(xb),
        "xT": xT,
        "waoT": waoT, "w1T": w1T, "w2T": w2T,
        "smallc": smallc, "trig": trig,
        "mask01": _mask01_tiles(),
    }
    in_maps = []
    for j in range(NCORES):
        wq = np.stack([
            np.ascontiguousarray(
                qkv_ws[s * D + 128 * j: s * D + 128 * j + 128].T
                .reshape(8, 128, 128))
            for s in range(3)])  # [3, k, p, c]
        wq = np.ascontiguousarray(wq.transpose(0, 2, 1, 3).astype(bf16))
        ub = np.stack([
            np.concatenate([u_qkv[s * D + 128 * j: s * D + 128 * j + 128]
                            for s in range(3)]),
            np.concatenate([b_qkv[s * D + 128 * j: s * D + 128 * j + 128]
                            for s in range(3)])])  # [2, 384]
        m = dict(common)
        m["wqkvT"] = wq  # [3, 128, 8, 128] = (s, p, k, c)
        m["ubrow"] = np.ascontiguousarray(ub.astype(bf16))
        m["xsliceT"] = np.ascontiguousarray(
            xT[:, :, TOK * j:TOK * j + TOK])
        in_maps.append(m)
    return in_maps


def _assemble(res):
    """Gather per-core feature-major outputs into the full [1, S, D]."""
    parts = []
    for j in range(NCORES):
        o = res.results[j]["out"]  # [8, 128, TOK] feature-major
        parts.append(np.ascontiguousarray(
            o.transpose(2, 0, 1).reshape(TOK, D)))
    return np.concatenate(parts, axis=0).reshape(1, S, D).astype(np.float32)


def kernel(**inputs):
    nc = _get_nc()
    in_maps = _prep_inputs(**inputs)
    res = run_bass_kernel_spmd(nc, in_maps, core_ids=list(range(NCORES)))
    return _assemble(res)


# revision 43
# speedup vs baseline: 1.3903x; 1.0917x over previous
"""DiT block with block-diffusion sparse attention on 8 Trainium2 NeuronCores.

v3 strategy:
  - adaLN modulation computed on HOST (tiny matvec); gamma scales folded into
    the QKV / MLP1 weights, shifts folded into bias rows.
  - LN1 never materializes h: QKV matmuls read host-pretransposed x^T
    directly; the per-token (-mu, sigma) correction enters the same PSUM as a
    rank-2 matmul with stationary (u, b) = ((W*gamma)@1, W@beta); the 1/sigma
    scale is folded into the RoPE cos/sin tables. Stats come from a
    token-major bn_stats pass.
  - Attention: 2 heads per core, scores in S^T orientation, both heads of an
    item share one PSUM tile / one exp / one mask op. Softmax normalization
    is DEFERRED: unnormalized o + denominator rows are evacuated per stream,
    reciprocals batched per half (keeps gpsimd/vector off the per-stream
    critical path).
  - PSUM is always evacuated through ScalarE (ACT) before VectorE touches the
    data - DVE reads from PSUM are ~4x slower than from SBUF.
  - One AllToAll converts head-sharded attention output to token-sharded.
  - attn_out, LN2, MLP token-sharded with full bf16 weights prefetched during
    the attention/A2A window. Output written feature-major; host transposes.
"""

import os
import numpy as np
import ml_dtypes

import concourse.bass as bass
import concourse.tile as tile
from concourse import bacc, mybir
from concourse.bass_utils import run_bass_kernel_spmd
from concourse.masks import make_identity

bf16 = ml_dtypes.bfloat16
fp8 = ml_dtypes.float8_e4m3
FP = mybir.dt.float32
BF = mybir.dt.bfloat16
F8 = mybir.dt.float8e4
AF = mybir.ActivationFunctionType
ALU = mybir.AluOpType
DR = mybir.MatmulPerfMode.DoubleRow
W1SCALE = 64.0
W2SCALE = 128.0

NCORES = 8
S, N, D, H, HD, BS, COND = 2048, 1024, 1024, 16, 64, 16, 128
TOK = S // NCORES  # 256 tokens per core after A2A


def _attn_schedule():
    """Per q-chunk list of (ktile, col0, col1, mask) in S^T orientation."""
    sched = []
    for c in range(4):
        items = []
        if c < 2:  # noisy q chunk
            for j in range(4 * c + 4):  # clean k tiles, bq > bk
                js = j - 4 * c
                if js < 0:
                    items.append((8 + j, 0, 512, None))
                else:
                    items.append((8 + j, 128 * js, 512, "strict"))
            for s in range(4):  # own-block diagonal (noisy k)
                items.append((4 * c + s, 128 * s, 128 * s + 128, "diag"))
        else:  # clean q chunk, bq >= bk
            cq = c - 2
            for j in range(4 * cq + 4):
                js = j - 4 * cq
                if js < 0:
                    items.append((8 + j, 0, 512, None))
                else:
                    items.append((8 + j, 128 * js, 512, "incl"))
        assert items[0][1] == 0 and items[0][2] == 512
        sched.append(items)
    return sched


MASK_OFF = {"diag": 0, "strict": 256, "incl": 512}


def build_program(single=False, dbg=False):
    """single=True builds a 1-device variant (A2A replaced by a local DMA
    copy) for TimelineSim cost-model analysis. dbg=True adds a debug output
    with intermediate tensors."""
    nc = bacc.Bacc(
        "TRN2", target_bir_lowering=False, debug=False,
        enable_asserts=False, num_devices=1 if single else NCORES,
    )

    def din(name, shape, dt=BF):
        return nc.dram_tensor(name, shape, dt, kind="ExternalInput").ap()

    x_d = din("x", [S, D])                            # token-major (stats)
    xT_d = din("xT", [8, 128, S])                     # feature-major (k, p, t)
    xsT_d = din("xsliceT", [8, 128, TOK])             # residual slice (k, p, t)
    trig_d = din("trig", [128, 2 * S])                # cos2 | sin2(dest-signed)
    mask01_d = din("mask01", [128, 768])              # diag|strict|incl x2
    wqkv_d = din("wqkvT", [3, 128, 8, 128])           # (s, p, k, c) g-scaled
    ub_d = din("ubrow", [2, 384])                     # (u; b) per-core slice
    wao_d = din("waoT", [2, 128, 4, 8, 128])          # (g, p, mi, k, c)
    w1_d = din("w1T", [8, 128, 4, 4, 2, 128], F8)     # (g, p, mi, j, i, c)
    w2_d = din("w2T", [8, 128, 16, 2, 128], F8)       # (m, p, j, i, c)
    smallc_d = din("smallc", [128, 64], FP)           # gmsa|gmlp128|b1'|b2|gb2
    out_d = nc.dram_tensor("out", [8, 128, TOK], FP, kind="ExternalOutput").ap()
    dbg_d = (nc.dram_tensor("dbg", [8, 128, S], BF,
                            kind="ExternalOutput").ap() if dbg else None)

    sched = _attn_schedule()

    with tile.TileContext(nc) as tc:
        with tc.tile_pool(name="const", bufs=1) as const, \
             tc.tile_pool(name="dram", bufs=1, space="DRAM") as dram, \
             tc.tile_pool(name="qkvr", bufs=1) as qkvr, \
             tc.tile_pool(name="vaugp", bufs=1) as vaugp, \
             tc.tile_pool(name="x2p", bufs=1) as x2p, \
             tc.tile_pool(name="gp", bufs=1) as gp:

            # ---------------- early DMAs (stats x first, then xT) ------
            xstat = tc.alloc_tile_pool(name="xstat", bufs=2)
            xTp = tc.alloc_tile_pool(name="xTp", bufs=1)
            x_r = x_d.rearrange("(t p) d -> p t d", p=128)  # [128,16,D]
            xg_sb = []
            for g in range(4):
                xg = xstat.tile([128, 4, D], BF, tag="x", name=f"xg{g}")
                nc.sync.dma_start(out=xg, in_=x_r[:, 4 * g:4 * g + 4, :])
                xg_sb.append(xg)
            xT_sb = [xTp.tile([128, S], BF, name=f"xT{k}") for k in range(8)]
            for k in range(8):
                nc.sync.dma_start(out=xT_sb[k], in_=xT_d[k])

            # ---------------- constants / small inputs ----------------
            trig_sb = const.tile([128, 2 * S], BF)
            nc.sync.dma_start(out=trig_sb, in_=trig_d)
            mask_sb = const.tile([128, 768], BF)
            nc.sync.dma_start(out=mask_sb, in_=mask01_d)
            smallc = const.tile([128, 64], FP)
            nc.sync.dma_start(out=smallc, in_=smallc_d)
            gmsa_sb = smallc[:, 0:8]
            gmlp_sb = smallc[:, 8:16]
            b1_sb = smallc[:, 16:48]
            b2_sb = smallc[:, 48:56]
            gb2_sb = smallc[:, 56:64]
            ub_sb = const.tile([2, 384], BF)
            nc.scalar.dma_start(out=ub_sb, in_=ub_d)
            ones_sb = const.tile([128, 1], BF)
            nc.vector.memset(ones_sb, 1.0)
            ones_row = const.tile([1, 128], BF)
            nc.vector.memset(ones_row, 1.0)
            eps128 = const.tile([128, 1], FP)
            nc.vector.memset(eps128, 1e-5)
            eps1 = const.tile([1, 1], FP)
            nc.vector.memset(eps1, 1e-5)
            ident_f = const.tile([128, 128], FP)
            make_identity(nc, ident_f)
            ident_b = const.tile([128, 128], BF)
            nc.vector.tensor_copy(out=ident_b, in_=ident_f)

            # residual slice (feature-major) straight from DRAM
            xsT = [x2p.tile([128, TOK], BF, name=f"xsT{j}") for j in range(8)]
            for fj in range(8):
                nc.sync.dma_start(out=xsT[fj], in_=xsT_d[fj])

            # qkv weights early on the scalar queue
            wqkvp = tc.alloc_tile_pool(name="wqkvp", bufs=1)
            wq_sb = [wqkvp.tile([128, 8 * 128], BF, name=f"wq{m}")
                     for m in range(3)]
            for m in range(3):
                nc.scalar.dma_start(
                    out=wq_sb[m].rearrange("p (k c) -> p k c", c=128),
                    in_=wqkv_d[m])

            # ---------------- phase 0: LN1 stats (token-major) ---------
            # per 128-token tile: bn stats -> (-mu, sd, rstd) columns of a
            # [128, 48] tile; one PE transpose + DRAM bounce turns them into
            # rows aligned with qT columns.
            stats_dr = dram.tile([3, S], BF)
            rows_sb = const.tile([2, S], BF)   # (negmu ; sd)
            rstd_row = const.tile([1, S], BF)
            with tc.tile_pool(name="statp", bufs=4) as statp, \
                 tc.tile_pool(name="st24", bufs=2) as st24p, \
                 tc.tile_pool(name="stps", bufs=2, space="PSUM") as stps:
                # pipelined per token-half: stats -> transpose -> bounce
                for half in range(2):
                    st24 = st24p.tile([128, 24], FP, tag="st24")
                    for g in range(2 * half, 2 * half + 2):
                        for sub in range(4):
                            ti = (4 * g + sub) % 8
                            x_sb = xg_sb[g][:, sub, :]
                            st = statp.tile([128, 2, 6], FP, tag="bst")
                            for sg in range(2):
                                nc.vector.bn_stats(
                                    out=st[:, sg, :],
                                    in_=x_sb[:, 512 * sg:512 * sg + 512])
                            mv = statp.tile([128, 2], FP, tag="mv")
                            nc.vector.bn_aggr(out=mv, in_=st)
                            nc.vector.tensor_scalar_mul(
                                st24[:, ti:ti + 1], mv[:, 0:1], -1.0)
                            nc.scalar.activation(
                                out=st24[:, 8 + ti:9 + ti], in_=mv[:, 1:2],
                                func=AF.Sqrt, bias=eps128, scale=1.0)
                            nc.vector.reciprocal(
                                out=st24[:, 16 + ti:17 + ti],
                                in_=st24[:, 8 + ti:9 + ti])
                    ps = stps.tile([24, 128], FP, tag="stt")
                    nc.tensor.transpose(ps, st24, ident_f)
                    st24b = st24p.tile([24, 128], BF, tag="st24b")
                    nc.vector.tensor_copy(out=st24b, in_=ps)
                    hs = slice(1024 * half, 1024 * half + 1024)
                    nc.gpsimd.dma_start(
                        out=stats_dr[:, hs].rearrange(
                            "v (t p) -> v t p", p=128),
                        in_=st24b)
                    nc.gpsimd.dma_start(out=rows_sb[:, hs],
                                        in_=stats_dr[0:2, hs])
                    nc.gpsimd.dma_start(out=rstd_row[:, hs],
                                        in_=stats_dr[2:3, hs])

            # rstd broadcast via PE rank-1 -> ACT evac -> fold into trig
            cosr = const.tile([128, S], BF)
            sinr = const.tile([128, S], BF)
            with tc.tile_pool(name="rbps", bufs=1, space="PSUM") as rbps, \
                 tc.tile_pool(name="rbt", bufs=1) as rbt:
                rstd_ps = rbps.tile([128, S], FP)
                for q in range(4):
                    nc.tensor.matmul(
                        rstd_ps[:, 512 * q:512 * q + 512], ones_row,
                        rstd_row[:, 512 * q:512 * q + 512],
                        start=True, stop=True, skip_group_check=True)
                rstd_bc = rbt.tile([128, S], BF)
                for q in range(2):
                    qs = slice(1024 * q, 1024 * q + 1024)
                    nc.scalar.copy(out=rstd_bc[:, qs], in_=rstd_ps[:, qs])
                    nc.vector.tensor_mul(cosr[:, qs], trig_sb[:, qs],
                                         rstd_bc[:, qs])
                    nc.vector.tensor_mul(
                        sinr[:, qs], trig_sb[:, S + 1024 * q:S + 1024 * q + 1024],
                        rstd_bc[:, qs])

            # ---------------- phase 1: QKV + RoPE ----------------------
            qT = qkvr.tile([128, S], BF)
            kT = qkvr.tile([128, S], BF)
            vT = qkvr.tile([128, S], BF)
            qkv_dst = [qT, kT, vT]

            with tc.tile_pool(name="ropep", bufs=3) as ropep, \
                 tc.tile_pool(name="qkvps", bufs=3, space="PSUM") as qkvps:
                for n in range(4):
                    nsl = slice(512 * n, 512 * n + 512)
                    for m in range(3):
                        ps = qkvps.tile([128, 512], FP, tag="qkvps")
                        for k in range(8):
                            nc.tensor.matmul(
                                ps, wq_sb[m][:, 128 * k:128 * k + 128],
                                xT_sb[k][:, nsl],
                                start=(k == 0), stop=False)
                        # rank-2: + u (.) (-mu)  +  b (.) sd
                        nc.tensor.matmul(
                            ps, ub_sb[:, 128 * m:128 * m + 128],
                            rows_sb[0:2, nsl], start=False, stop=True,
                            skip_group_check=True)
                        # ACT evac, then rope on SBUF bf16:
                        # dst = pb*cosr + swap32(pb)*sinr  (sign in sinr)
                        pb = ropep.tile([128, 512], BF, tag="pb")
                        nc.scalar.copy(out=pb, in_=ps)
                        pbs = ropep.tile([128, 512], BF, tag="pbs")
                        for h in range(2):
                            r = 64 * h
                            nc.vector.tensor_copy(
                                out=pbs[r:r + 32, :], in_=pb[r + 32:r + 64, :])
                            nc.vector.tensor_copy(
                                out=pbs[r + 32:r + 64, :], in_=pb[r:r + 32, :])
                        t1 = ropep.tile([128, 512], BF, tag="t1")
                        nc.vector.tensor_mul(t1, pb, cosr[:, nsl])
                        t2 = ropep.tile([128, 512], BF, tag="t2")
                        nc.vector.tensor_mul(t2, pbs, sinr[:, nsl])
                        nc.vector.tensor_add(
                            qkv_dst[m][:, nsl], t1, t2)
            wqkvp.release()
            xTp.release()
            xstat.release()

            # ---------------- phase 2: V token-major (+ones col) -------
            vaug = [vaugp.tile([128, 130], BF, name=f"vaug{kt}")
                    for kt in range(16)]
            with tc.tile_pool(name="vtps", bufs=2, space="PSUM") as vtps:
                for kt in range(16):
                    ps = vtps.tile([128, 128], BF, tag="vt")
                    nc.tensor.transpose(
                        ps, vT[:, 128 * kt:128 * kt + 128], ident_b)
                    va = vaug[kt]
                    nc.vector.memset(va[:, 64:65], 1.0)
                    nc.vector.memset(va[:, 129:130], 1.0)
                    nc.scalar.copy(
                        out=va[:, 0:130].rearrange(
                            "p (h y) -> p h y", y=65)[:, :, 0:64],
                        in_=ps.rearrange("p (h d) -> p h d", d=64))

            # ---------------- weight prefetch (runs under attn + A2A) --
            waop = tc.alloc_tile_pool(name="waop", bufs=2)
            w1p = tc.alloc_tile_pool(name="w1p", bufs=4)
            w2p = tc.alloc_tile_pool(name="w2p", bufs=3)
            wao_sb = [waop.tile([128, 4 * 8 * 128], BF, tag="wao",
                                name=f"wao{g}")
                      for g in range(2)]
            for g in range(2):
                nc.scalar.dma_start(
                    out=wao_sb[g].rearrange("p (mi k c) -> p mi k c",
                                            k=8, c=128),
                    in_=wao_d[g])
            w1_sb = [w1p.tile([128, 4 * 4 * 2 * 128], F8, tag="w1",
                              name=f"w1_{g}")
                     for g in range(8)]
            for g in range(8):
                nc.scalar.dma_start(
                    out=w1_sb[g].rearrange("p (mi j i c) -> p mi j i c",
                                           j=4, i=2, c=128),
                    in_=w1_d[g])
            w2_sb = [w2p.tile([128, 16 * 2 * 128], F8, tag="w2",
                              name=f"w2_{m}")
                     for m in range(8)]
            for m in range(8):
                nc.scalar.dma_start(
                    out=w2_sb[m].rearrange("p (j i c) -> p j i c",
                                           i=2, c=128),
                    in_=w2_d[m])

            # ---------------- phase 3: sparse attention ----------------
            # Both heads of an item share one [128, 2, 512] score PSUM tile,
            # one exp, one mask op. Normalization deferred: o_un + den rows
            # evacuated per stream; reciprocal batched per half.
            onorm = [qkvr.tile([128, N], BF, name=f"onorm{hh}")
                     for hh in range(2)]
            obounce = dram.tile([NCORES, 128, TOK], BF)
            orecvb = dram.tile([NCORES, 128, TOK], BF)
            orecv = x2p.tile([128, 8 * TOK], BF)
            o_un = [qkvr.tile([64, 512], BF, name=f"oun{k}")
                    for k in range(8)]
            # den rows live at partitions {0,32,64,96} (safe write offsets);
            # memset 1.0 so the reciprocal over unused partitions is benign
            den4 = [qkvr.tile([128, 512], FP, name=f"den{hh}")
                    for hh in range(2)]
            recip4 = [qkvr.tile([128, 512], BF, name=f"recip{hh}")
                      for hh in range(2)]
            for hh in range(2):
                nc.vector.memset(den4[hh], 1.0)
            with tc.tile_pool(name="sps", bufs=2, space="PSUM") as sps, \
                 tc.tile_pool(name="ops", bufs=4, space="PSUM") as ops, \
                 tc.tile_pool(name="ptp", bufs=4) as ptp, \
                 tc.tile_pool(name="nrm", bufs=2) as nrm:
                for c in range(4):
                    items = sched[c]
                    q0 = 512 * c
                    o_ps = [ops.tile([65, 512], FP, tag="ops",
                                     name=f"ops{c}_{h}") for h in range(2)]
                    for idx, (kt, c0, c1, mk) in enumerate(items):
                        w = c1 - c0
                        s_ps = sps.tile([128, 2, 512], FP, tag="sps")
                        for h in range(2):
                            nc.tensor.matmul(
                                s_ps[:, h, 0:w],
                                kT[64 * h:64 * h + 64,
                                   128 * kt:128 * kt + 128],
                                qT[64 * h:64 * h + 64, q0 + c0:q0 + c1],
                                start=True, stop=True, skip_group_check=True)
                        p_sb = ptp.tile([128, 2, 512], BF, tag="pt")
                        if w == 512:
                            nc.scalar.activation(out=p_sb[:, :, :],
                                                 in_=s_ps[:, :, :],
                                                 func=AF.Exp, scale=0.125)
                        else:
                            for h in range(2):
                                nc.scalar.activation(out=p_sb[:, h, 0:w],
                                                     in_=s_ps[:, h, 0:w],
                                                     func=AF.Exp, scale=0.125)
                        if mk is not None:
                            mo = MASK_OFF[mk]
                            for h in range(2):
                                nc.gpsimd.tensor_mul(
                                    p_sb[:, h, 0:128], p_sb[:, h, 0:128],
                                    mask_sb[:, mo + 128 * h:mo + 128 * h + 128])
                        for h in range(2):
                            nc.tensor.matmul(
                                o_ps[h][:, c0:c1],
                                vaug[kt][:, 65 * h:65 * h + 65],
                                p_sb[:, h, 0:w], start=(idx == 0),
                                stop=(idx == len(items) - 1),
                                skip_group_check=True)
                    for h in range(2):
                        k = 2 * c + h
                        r = 32 * (k % 4)
                        nc.scalar.copy(out=o_un[k], in_=o_ps[h][0:64, :])
                        nc.scalar.copy(out=den4[c // 2][r:r + 1, :],
                                       in_=o_ps[h][64:65, :])
                    # reciprocal as soon as a half's denominators are done
                    # (DVE is otherwise idle here); the rest of the
                    # normalization is deferred past all items so the ACT /
                    # gpsimd queues never convoy the next chunk's exps/masks
                    if c % 2 == 1:
                        hh = c // 2
                        with nc.allow_low_precision(reason="softmax denom "
                                                    "recip as bf16"):
                            nc.vector.reciprocal(out=recip4[hh],
                                                 in_=den4[hh])
                # deferred normalization tail + staging
                for k in range(8):
                    hh, kk = k // 4, k % 4
                    cc, h = k // 2, k % 2
                    # partition_broadcast reads partition 0 of the TILE
                    # (AP partition offset ignored): stage the row first
                    rtmp = nrm.tile([1, 512], BF, tag="rtmp", bufs=2)
                    nc.scalar.copy(
                        out=rtmp, in_=recip4[hh][32 * kk:32 * kk + 1, :])
                    rbc = nrm.tile([64, 512], BF, tag="rbc", bufs=3)
                    nc.gpsimd.partition_broadcast(rbc, rtmp)
                    nc.vector.tensor_mul(
                        onorm[hh][64 * h:64 * h + 64,
                                  (512 * cc) % N:(512 * cc) % N + 512],
                        o_un[k], rbc)
                    if k % 4 == 3:
                        hh = k // 4
                        nc.sync.dma_start(
                            out=obounce[4 * hh:4 * hh + 4].rearrange(
                                "j p t -> p j t"),
                            in_=onorm[hh].rearrange("p (j t) -> p j t", t=TOK))

            if dbg_d is not None:
                nc.sync.dma_start(out=dbg_d[0], in_=qT)
                nc.sync.dma_start(out=dbg_d[1], in_=kT)
                nc.sync.dma_start(out=dbg_d[2], in_=vT)
                nc.sync.dma_start(out=dbg_d[3][:, 0:N], in_=onorm[0])
                nc.sync.dma_start(out=dbg_d[4][:, 0:N], in_=onorm[1])
                nc.sync.dma_start(out=dbg_d[5][0:64, 0:512], in_=o_un[0])
                nc.sync.dma_start(out=dbg_d[5][0:64, 512:1024], in_=o_un[1])
                nc.sync.dma_start(out=dbg_d[5][0:64, 1024:1536], in_=o_un[6])
                nc.sync.dma_start(out=dbg_d[5][0:64, 1536:2048], in_=o_un[7])
                dbgrc = qkvr.tile([128, 1024], BF)
                nc.vector.tensor_copy(out=dbgrc[:, 0:512], in_=den4[0])
                nc.vector.tensor_copy(out=dbgrc[:, 512:1024], in_=den4[1])
                nc.sync.dma_start(out=dbg_d[6][:, 0:1024], in_=dbgrc)
                nc.sync.dma_start(out=dbg_d[6][:, 1024:1536], in_=recip4[0])
                nc.sync.dma_start(out=dbg_d[6][:, 1536:2048], in_=recip4[1])
            if single:
                nc.sync.dma_start(out=orecvb[:], in_=obounce[:])
            else:
                nc.gpsimd.collective_compute(
                    "AllToAll", ALU.bypass,
                    replica_groups=[list(range(NCORES))],
                    ins=[obounce.opt()], outs=[orecvb.opt()])
            nc.sync.dma_start(
                out=orecv.rearrange("p (j t) -> p j t", t=TOK),
                in_=orecvb.rearrange("j p t -> p j t"))
            if dbg_d is not None:
                nc.sync.dma_start(out=dbg_d[7], in_=orecv)

            # ---------------- phase 4: attn_out + residual -------------
            x2T = [x2p.tile([128, TOK], FP, name=f"x2T{m}") for m in range(8)]
            x2b = [x2p.tile([128, TOK], BF, name=f"x2b{m}") for m in range(8)]
            sqb = [x2p.tile([128, TOK], BF, name=f"sqb{m}") for m in range(8)]
            with tc.tile_pool(name="aops", bufs=3, space="PSUM") as aops, \
                 tc.tile_pool(name="aot", bufs=3) as aot:
                for g in range(2):
                    for mi in range(4):
                        m = 4 * g + mi
                        ps = aops.tile([128, TOK], FP, tag="aops")
                        for k in range(8):
                            off = 1024 * mi + 128 * k
                            nc.tensor.matmul(
                                ps, wao_sb[g][:, off:off + 128],
                                orecv[:, TOK * k:TOK * k + TOK],
                                start=(k == 0), stop=(k == 7))
                        ao_sb = aot.tile([128, TOK], FP, tag="ao")
                        nc.scalar.copy(out=ao_sb, in_=ps)
                        nc.vector.scalar_tensor_tensor(
                            out=x2T[m], in0=ao_sb,
                            scalar=gmsa_sb[:, m:m + 1],
                            in1=xsT[m], op0=ALU.mult, op1=ALU.add)
                        nc.vector.tensor_copy(out=x2b[m], in_=x2T[m])
                        nc.vector.tensor_mul(sqb[m], x2b[m], x2b[m])

            # ---------------- phase 5: LN2 (gamma/beta folded on host) -
            # h2 written as fp8 DoubleRow pairs: h2dr[j][:, i, :] = k-tile 2j+i
            h2dr = [x2p.tile([128, 2, TOK], F8, name=f"h2dr{j}")
                    for j in range(4)]
            with tc.tile_pool(name="l2ps", bufs=1, space="PSUM") as l2ps, \
                 tc.tile_pool(name="l2t", bufs=1) as l2t:
                sum_ps = l2ps.tile([1, TOK], FP, tag="l2sum")
                for k in range(8):
                    nc.tensor.matmul(sum_ps, ones_sb, x2b[k],
                                     start=(k == 0), stop=(k == 7))
                ssq_ps = l2ps.tile([1, TOK], FP, tag="l2ssq")
                for k in range(8):
                    nc.tensor.matmul(ssq_ps, ones_sb, sqb[k],
                                     start=(k == 0), stop=(k == 7),
                                     skip_group_check=True)
                mu2 = l2t.tile([1, TOK], BF)
                nc.vector.tensor_scalar_mul(mu2, sum_ps, 1.0 / D)
                mu2f = l2t.tile([1, TOK], FP)
                nc.vector.tensor_scalar_mul(mu2f, sum_ps, 1.0 / D)
                var2 = l2t.tile([1, TOK], FP)
                musq = l2t.tile([1, TOK], FP)
                nc.vector.tensor_mul(musq, mu2f, mu2f)
                nc.vector.tensor_scalar_mul(var2, ssq_ps, 1.0 / D)
                nc.vector.tensor_sub(var2, var2, musq)
                sd2 = l2t.tile([1, TOK], FP)
                nc.scalar.activation(out=sd2, in_=var2, func=AF.Sqrt,
                                     bias=eps1, scale=1.0)
                rstd2 = l2t.tile([1, TOK], BF)
                with nc.allow_low_precision(reason="rstd2 row as bf16 "
                                            "matmul-broadcast operand"):
                    nc.vector.reciprocal(out=rstd2, in_=sd2)
                # row broadcasts via PE rank-1 + ACT evac
                mu2bc_ps = l2ps.tile([128, TOK], FP, tag="l2mub")
                nc.tensor.matmul(mu2bc_ps, ones_row, mu2,
                                 start=True, stop=True,
                                 skip_group_check=True)
                rstd2bc_ps = l2ps.tile([128, TOK], FP, tag="l2rsb")
                nc.tensor.matmul(rstd2bc_ps, ones_row, rstd2,
                                 start=True, stop=True,
                                 skip_group_check=True)
                mu2bc = l2t.tile([128, TOK], FP)
                nc.scalar.copy(out=mu2bc, in_=mu2bc_ps)
                rstd2bc = l2t.tile([128, TOK], FP)
                nc.scalar.copy(out=rstd2bc, in_=rstd2bc_ps)
                for k in range(8):
                    u = l2t.tile([128, TOK], FP, tag="u", bufs=2)
                    nc.vector.tensor_sub(u, x2T[k], mu2bc)
                    nc.vector.tensor_mul(h2dr[k // 2][:, k % 2, :],
                                         u, rstd2bc)

            # ---------------- phase 6: MLP (fp8 DoubleRow) -------------
            g_dr = [gp.tile([128, 2, TOK], F8, name=f"g{j}")
                    for j in range(16)]
            w1r = [w1_sb[g].rearrange("p (mi j i c) -> p mi j i c",
                                      j=4, i=2, c=128) for g in range(8)]
            with tc.tile_pool(name="m1ps", bufs=3, space="PSUM") as m1ps:
                for g in range(8):
                    for mi in range(4):
                        m = 4 * g + mi
                        ps = m1ps.tile([128, TOK], FP, tag="m1")
                        for j in range(4):
                            nc.tensor.matmul(ps, w1r[g][:, mi, j],
                                             h2dr[j], perf_mode=DR,
                                             start=(j == 0), stop=(j == 3))
                        gfunc = (AF.Identity if os.environ.get("DBG_NO_GELU")
                                 else AF.Gelu_apprx_tanh)
                        # psum holds W1SCALE * z: gelu(z + b1) via ACT scale
                        nc.scalar.activation(out=g_dr[m // 2][:, m % 2, :],
                                             in_=ps, func=gfunc,
                                             bias=b1_sb[:, m:m + 1],
                                             scale=1.0 / W1SCALE)

            w2r = [w2_sb[m].rearrange("p (j i c) -> p j i c", i=2, c=128)
                   for m in range(8)]
            with tc.tile_pool(name="m2ps", bufs=3, space="PSUM") as m2ps, \
                 tc.tile_pool(name="outp", bufs=3) as outp:
                for m in range(8):
                    ps = m2ps.tile([128, TOK], FP, tag="m2")
                    for j in range(16):
                        nc.tensor.matmul(ps, w2r[m][:, j], g_dr[j],
                                         perf_mode=DR,
                                         start=(j == 0), stop=(j == 15))
                    # psum = W2SCALE*(m - b2); evac: gmlp/W2SCALE * ps + gb2
                    mo = outp.tile([128, TOK], FP, tag="mo")
                    nc.scalar.activation(out=mo, in_=ps, func=AF.Identity,
                                         bias=gb2_sb[:, m:m + 1],
                                         scale=gmlp_sb[:, m:m + 1])
                    outT = outp.tile([128, TOK], FP, tag="outT")
                    nc.vector.tensor_add(outT, mo, x2T[m])
                    nc.sync.dma_start(out=out_d[m], in_=outT)

            w2p.release()
            w1p.release()
            waop.release()

    nc.compile()
    return nc


# ---------------------------------------------------------------------------
# host side
# ---------------------------------------------------------------------------

_NC = None


def _get_nc():
    global _NC
    if _NC is None:
        _NC = build_program()
    return _NC


def _mask01_tiles():
    """[128,128] multiplicative 0/1 masks in S^T orientation (rows=k,
    cols=q), each doubled for the 2-head layout: [diag x2 | strict x2 |
    incl x2]."""
    a = np.arange(128) // BS
    diag = (a[:, None] == a[None, :])
    strict = (a[None, :] > a[:, None])
    incl = (a[None, :] >= a[:, None])
    m = np.concatenate([diag, diag, strict, strict, incl, incl],
                       axis=1).astype(np.float32)
    return np.ascontiguousarray(m.astype(bf16))


def _tile4(wT, km, mm):
    """[K, M] -> (m, p, k, c) with arr[m, p, k, c] = wT[128k+p, 128m+c]."""
    return wT.reshape(km, 128, mm, 128).transpose(2, 1, 0, 3)


def _group(w4, gs):
    """(m, p, k, c) -> (g, p, m_in_g, k, c) groups of gs m-tiles."""
    mm, p, km, c = w4.shape
    return np.ascontiguousarray(
        w4.reshape(mm // gs, gs, p, km, c).transpose(0, 2, 1, 3, 4)
        .astype(bf16))


def _prep_inputs(x, c, cos, sin, norm1_w, qkv_w, attn_out_w, norm2_w,
                 mlp_w1, mlp_b1, mlp_w2, mlp_b2, adaLN_w, adaLN_b):
    f32 = np.float32
    x = np.asarray(x, f32).reshape(S, D)
    c = np.asarray(c, f32).reshape(COND)
    cos = np.asarray(cos, f32)
    sin = np.asarray(sin, f32)
    qkv_w = np.asarray(qkv_w, f32)
    mlp_w1 = np.asarray(mlp_w1, f32)

    # adaLN modulation on host
    mods = adaLN_w.astype(f32) @ c + np.asarray(adaLN_b, f32)
    sh_msa, sc_msa, g_msa, sh_mlp, sc_mlp, g_mlp = mods.reshape(6, D)

    gam1 = (1.0 + sc_msa) * np.asarray(norm1_w, f32)          # [D]
    qkv_ws = qkv_w * gam1[None, :]                            # [3D, D]
    u_qkv = qkv_ws.sum(axis=1)                                # [3D]
    b_qkv = qkv_w @ sh_msa                                    # [3D]

    gam2 = (1.0 + sc_mlp) * np.asarray(norm2_w, f32)          # [D]
    w1s = mlp_w1 * gam2[None, :]                              # [4D, D]
    b1f = np.asarray(mlp_b1, f32) + mlp_w1 @ sh_mlp           # [4D]
    b2 = np.asarray(mlp_b2, f32)

    xb = x.astype(bf16)
    xT = np.ascontiguousarray(
        xb.T.reshape(8, 128, S))                              # (k, p, t)

    # rope tables expanded to S columns; sin table is DEST-signed for the
    # pure-swap pbs layout: rows 0:32 get -sin (they receive p[32:64]),
    # rows 32:64 get +sin.
    cs = np.concatenate([cos, cos], axis=-1).T                # [64, N]
    ss = np.concatenate([-sin.T, sin.T], axis=0)              # [64, N]
    cos2 = np.tile(np.vstack([cs, cs]), (1, 2))               # [128, S]
    sin2 = np.tile(np.vstack([ss, ss]), (1, 2))               # [128, S]
    trig = np.ascontiguousarray(np.hstack([cos2, sin2]).astype(bf16))

    waoT = _group(_tile4(np.asarray(attn_out_w, f32).T, 8, 8), 4)
    # fp8 DoubleRow weights, scaled into e4m3 range (240 max on TRN)
    w1q = np.clip(_tile4(w1s.T, 8, 32) * W1SCALE, -240, 240)  # [32,128,8,128]
    w1T = np.ascontiguousarray(
        w1q.reshape(8, 4, 128, 4, 2, 128).transpose(0, 2, 1, 3, 4, 5)
        .astype(fp8))                               # (g, p, mi, j, i, c)
    w2q = np.clip(_tile4(np.asarray(mlp_w2, f32).T, 32, 8) * W2SCALE,
                  -240, 240)                        # [8, 128, 32, 128]
    w2T = np.ascontiguousarray(
        w2q.reshape(8, 128, 16, 2, 128).astype(fp8))  # (m, p, j, i, c)

    smallc = np.ascontiguousarray(np.hstack([
        g_msa.reshape(8, 128).T,
        (g_mlp / W2SCALE).reshape(8, 128).T,
        b1f.reshape(32, 128).T,
        b2.reshape(8, 128).T,
        (g_mlp * b2).reshape(8, 128).T]).astype(f32))         # [128, 64]

    common = {
        "x": np.ascontiguousarray(xb),
        "xT": xT,
        "waoT": waoT, "w1T": w1T, "w2T": w2T,
        "smallc": smallc, "trig": trig,
        "mask01": _mask01_tiles(),
    }
    in_maps = []
    for j in range(NCORES):
        wq = np.stack([
            np.ascontiguousarray(
                qkv_ws[s * D + 128 * j: s * D + 128 * j + 128].T
                .reshape(8, 128, 128))
            for s in range(3)])  # [3, k, p, c]
        wq = np.ascontiguousarray(wq.transpose(0, 2, 1, 3).astype(bf16))
        ub = np.stack([
            np.concatenate([u_qkv[s * D + 128 * j: s * D + 128 * j + 128]
                            for s in range(3)]),
            np.concatenate([b_qkv[s * D + 128 * j: s * D + 128 * j + 128]
                            for s in range(3)])])  # [2, 384]
        m = dict(common)
        m["wqkvT"] = wq  # [3, 128, 8, 128] = (s, p, k, c)
        m["ubrow"] = np.ascontiguousarray(ub.astype(bf16))
        m["xsliceT"] = np.ascontiguousarray(
            xT[:, :, TOK * j:TOK * j + TOK])
        in_maps.append(m)
    return in_maps


def _assemble(res):
    """Gather per-core feature-major outputs into the full [1, S, D]."""
    parts = []
    for j in range(NCORES):
        o = res.results[j]["out"]  # [8, 128, TOK] feature-major
        parts.append(np.ascontiguousarray(
            o.transpose(2, 0, 1).reshape(TOK, D)))
    return np.concatenate(parts, axis=0).reshape(1, S, D).astype(np.float32)


def kernel(**inputs):
    nc = _get_nc()
    in_maps = _prep_inputs(**inputs)
    res = run_bass_kernel_spmd(nc, in_maps, core_ids=list(range(NCORES)))
    return _assemble(res)
